# revision 14
# baseline (speedup 1.0000x reference)
"""Trainium2 Bass kernel for nn_AnotherMamba (selective-scan Mamba block).

Sharding: 8 cores = 2 (batch) x 4 (d_inner chunks of 1024 channels).
v2 design: DVE does only scans + m2 products; B/C broadcasts go through
DMA (0-stride gather APs) into bf16 SBUF; b_in products + reduce level 1
run on GpSimd; depthwise conv runs on the PE as 4 accumulating diagonal
matmuls; all activations (dA exps, silu/softplus chains) on Scalar.
Host sums the 4 channel-shard partials per batch.
"""
import os
import sys

import numpy as np

sys.path.insert(0, "/opt/trn_rl_repo")

import ml_dtypes  # noqa: E402

BF = ml_dtypes.bfloat16

# Problem dims (hardcoded per harness contract)
B_, L_, DM, DIN, DXB, DS, DC, DTR = 2, 4096, 2048, 4096, 1024, 16, 4, 128
NCORES = 8
NCH = 4                  # d_inner chunk cores per batch
DIN_SH = DIN // NCH      # 1024 channels per core
DXB_SH = DXB // NCH      # 256 B/x rows per core
NCT = DIN_SH // 128      # 8 channel tiles per core

# consts layout (128, NC) f32 columns
_A0 = 0                  # A: cols [0,128): col i*16+s = A[i*128+p, s]
_D0 = 128                # D: 8 cols
_CB0 = 136               # conv_b: 8 cols
_CBN = 144               # -conv_b: 8 cols
_BDT2 = 152              # 2*bdt: 8 cols
_BZ0 = 160               # bz: 8 cols
_BZN = 168               # -bz: 8 cols
_BX0 = 176               # bx in rows 0..31: col 176+i = bx[i*32:(i+1)*32]
_NC = 184


def _build(L, T):
    from concourse import bass, mybir
    from concourse.tile import TileContext

    F32, BF16 = mybir.dt.float32, mybir.dt.bfloat16
    AF = mybir.ActivationFunctionType
    OP = mybir.AluOpType
    NT = L // T
    NCK = 4              # state chunks of 4

    nc = bass.Bass()
    hsT = nc.declare_dram_parameter("hsT", [DM, L], BF16, isOutput=False)
    wxbT = nc.declare_dram_parameter("wxbT", [DM, 2 * DXB_SH], BF16, isOutput=False)
    wzS = nc.declare_dram_parameter("wzS", [NCT * 128, 16 * 128], BF16, isOutput=False)
    wcS = nc.declare_dram_parameter("wcS", [NCT * 128, 16 * 128], BF16, isOutput=False)
    wddT = nc.declare_dram_parameter("wddT", [DM, DTR], BF16, isOutput=False)
    wdtT = nc.declare_dram_parameter("wdtT", [DTR, DIN_SH], BF16, isOutput=False)
    woutT = nc.declare_dram_parameter("woutT", [DIN_SH, DM], BF16, isOutput=False)
    convd = nc.declare_dram_parameter("convd", [128, NCT * 4 * 128], BF16, isOutput=False)
    consts = nc.declare_dram_parameter("consts", [128, _NC], F32, isOutput=False)
    e32 = nc.declare_dram_parameter("e32", [32, 128], BF16, isOutput=False)
    outp = nc.declare_dram_parameter("outp", [DM, L], BF16, isOutput=True)

    with TileContext(nc) as tc:
        with tc.tile_pool(name="wp", bufs=1) as wp, \
             tc.tile_pool(name="wzp", bufs=2) as wzp, \
             tc.tile_pool(name="hsp", bufs=2) as hsp, \
             tc.tile_pool(name="sp", bufs=2) as sp, \
             tc.tile_pool(name="sq", bufs=2) as sq, \
             tc.tile_pool(name="ck", bufs=2) as ck, \
             tc.tile_pool(name="ck1", bufs=1) as ck1, \
             tc.tile_pool(name="rp", bufs=1) as rp, \
             tc.tile_pool(name="big", bufs=1) as big, \
             tc.tile_pool(name="psA", bufs=2, space="PSUM") as psA, \
             tc.tile_pool(name="psC", bufs=2, space="PSUM") as psC, \
             tc.tile_pool(name="psO", bufs=2, space="PSUM") as psO:

            # ---- resident weights / constants
            w_wxb = wp.tile([128, 16, 2 * DXB_SH], BF16, tag="w_wxb")
            w_wdd = wp.tile([128, 16, DTR], BF16, tag="w_wdd")
            w_wdt = wp.tile([128, DIN_SH], BF16, tag="w_wdt")
            w_wo = wp.tile([128, NCT, DM], BF16, tag="w_wo")
            w_cvd = wp.tile([128, NCT * 4, 128], BF16, tag="w_cvd")
            cst = wp.tile([128, _NC], F32, tag="cst")
            e32_sb = wp.tile([32, 128], BF16, tag="e32")
            nc.sync.dma_start(out=e32_sb[:], in_=e32[:])

            nc.sync.dma_start(out=w_wxb[:], in_=wxbT.rearrange("(kt p) c -> p kt c", p=128))
            nc.sync.dma_start(out=w_wdd[:], in_=wddT.rearrange("(kt p) c -> p kt c", p=128))
            nc.sync.dma_start(out=w_wdt[:], in_=wdtT[:])
            nc.sync.dma_start(out=w_wo[:], in_=woutT.rearrange("(ki p) m -> p ki m", p=128))
            nc.sync.dma_start(out=w_cvd[:], in_=convd.rearrange("p (i c) -> p i c", c=128))
            nc.sync.dma_start(out=cst[:], in_=consts[:])

            # persistent state
            carry = wp.tile([128, DIN_SH // 128 * 16], F32, tag="carry")  # (128, 128)
            xbprev = wp.tile([128, NCT * 4], BF16, tag="xbprev")
            nc.vector.memset(carry[:], 0.0)

            hsT_r = hsT.rearrange("(kt p) t -> p kt t", p=128)

            def col(c0, i, n=1):
                return cst[:, c0 + i:c0 + i + n]

            for k in range(NT):
                hs = hsp.tile([128, 16, T], BF16, tag="hs")
                nc.sync.dma_start(out=hs[:], in_=hsT_r[:, :, k * T:(k + 1) * T])

                # dtlow = hs @ Wdt_down.T  -> (128, T)
                ps_dl = psA.tile([128, T], F32, tag="psA")
                for kt in range(16):
                    nc.tensor.matmul(ps_dl[:], lhsT=w_wdd[:, kt, :], rhs=hs[:, kt, :],
                                     start=(kt == 0), stop=(kt == 15))
                dl = sq.tile([128, T], BF16, tag="dl")
                nc.scalar.copy(out=dl[:], in_=ps_dl[:])

                yf = big.tile([128, NCT, T], BF16, tag="yf")

                for i in range(NCT):
                    # stream this ctile's z and C weights
                    w_z = wzp.tile([128, 16, 128], BF16, tag="w_z")
                    nc.sync.dma_start(out=w_z[:],
                                      in_=wzS.rearrange("(i p) c -> i p c", p=128)[i])
                    w_c = wzp.tile([128, 16, 128], BF16, tag="w_c")
                    nc.sync.dma_start(out=w_c[:],
                                      in_=wcS.rearrange("(i p) c -> i p c", p=128)[i])

                    # ---- x+B combined projection (64 rows: x 0:32, B 32:64)
                    ps_x = psA.tile([64, T], F32, tag="psA")
                    for kt in range(16):
                        nc.tensor.matmul(ps_x[:], lhsT=w_wxb[:, kt, i * 64:(i + 1) * 64],
                                         rhs=hs[:, kt, :], start=(kt == 0), stop=(kt == 15))
                    xbs = sq.tile([64, T], BF16, tag="xbs")
                    nc.scalar.activation(xbs[0:32, :], ps_x[0:32, :], AF.Identity,
                                         bias=cst[0:32, _BX0 + i:_BX0 + i + 1], scale=1.0)
                    nc.scalar.copy(out=xbs[32:64, :], in_=ps_x[32:64, :])

                    # xb expansion via e32 matmul (PSUM) + scalar copy
                    ps_xb = psC.tile([128, T], F32, tag="psXB")
                    nc.tensor.matmul(ps_xb[:], lhsT=e32_sb[:], rhs=xbs[0:32, :],
                                     start=True, stop=True)
                    xb = sp.tile([128, T + 4], BF16, tag="xb")
                    if k == 0:
                        nc.vector.memset(xb[:, 0:4], 0.0)
                    else:
                        nc.vector.tensor_copy(xb[:, 0:4], xbprev[:, i * 4:(i + 1) * 4])
                    nc.scalar.copy(out=xb[:, 4:T + 4], in_=ps_xb[:])
                    nc.vector.tensor_copy(xbprev[:, i * 4:(i + 1) * 4], xb[:, T:T + 4])
                    # repack B rows state-major: b3 row (bh*4+cc) = 4T run
                    xs_ap = xbs[:]
                    xps = xs_ap.ap[0][0]
                    b3 = rp.tile([8, 4 * T], BF16, tag="b3")
                    b3ps = b3[:].ap[0][0]
                    nc.sync.dma_start(
                        out=bass.AP(tensor=b3[:].tensor, offset=b3[:].offset,
                                    ap=[[b3ps, 8], [T, 4], [1, T]]),
                        in_=bass.AP(tensor=xs_ap.tensor,
                                    offset=xs_ap.offset + 32 * xps,
                                    ap=[[xps, 32], [1, T]]))

                    # ---- z projection + C projection
                    ps_z = psA.tile([128, T], F32, tag="psA")
                    for kt in range(16):
                        nc.tensor.matmul(ps_z[:], lhsT=w_z[:, kt, :],
                                         rhs=hs[:, kt, :], start=(kt == 0), stop=(kt == 15))
                    # delta = softplus(dt_raw + 2*bdt)
                    ps_d = psA.tile([128, T], F32, tag="psA")
                    nc.tensor.matmul(ps_d[:], lhsT=w_wdt[:, i * 128:(i + 1) * 128],
                                     rhs=dl[:], start=True, stop=True)
                    ps_c = psA.tile([128, T], F32, tag="psA")
                    for kt in range(16):
                        nc.tensor.matmul(ps_c[:], lhsT=w_c[:, kt, :],
                                         rhs=hs[:, kt, :], start=(kt == 0), stop=(kt == 15))
                    # ---- conv via 4 accumulating diagonal matmuls
                    ps_cv = psC.tile([128, T], F32, tag="psC")
                    for j in range(4):
                        nc.tensor.matmul(ps_cv[:], lhsT=w_cvd[:, i * 4 + j, :],
                                         rhs=xb[:, 1 + j:1 + j + T],
                                         start=(j == 0), stop=(j == 3))

                    # ---- u = silu(cv + conv_b) via exp/ln (one table set)
                    e1u = sq.tile([128, T], BF16, tag="e1")
                    nc.scalar.activation(e1u[:], ps_cv[:], AF.Exp,
                                         bias=col(_CBN, i), scale=-1.0)
                    spu = sq.tile([128, T], BF16, tag="spl")
                    nc.scalar.activation(spu[:], e1u[:], AF.Ln, bias=1.0, scale=1.0)
                    sgu = sq.tile([128, T], BF16, tag="sg")
                    nc.scalar.activation(sgu[:], spu[:], AF.Exp, bias=0.0, scale=-1.0)
                    u = sp.tile([128, T], BF16, tag="u")
                    nc.vector.scalar_tensor_tensor(out=u[:], in0=ps_cv[:], scalar=col(_CB0, i),
                                                   in1=sgu[:], op0=OP.add, op1=OP.mult)

                    # ---- sz = silu(z + bz)
                    e1z = sq.tile([128, T], BF16, tag="e1")
                    nc.scalar.activation(e1z[:], ps_z[:], AF.Exp,
                                         bias=col(_BZN, i), scale=-1.0)
                    spz = sq.tile([128, T], BF16, tag="spl")
                    nc.scalar.activation(spz[:], e1z[:], AF.Ln, bias=1.0, scale=1.0)
                    sgz = sq.tile([128, T], BF16, tag="sg")
                    nc.scalar.activation(sgz[:], spz[:], AF.Exp, bias=0.0, scale=-1.0)
                    sz = sp.tile([128, T], BF16, tag="sz")
                    nc.vector.scalar_tensor_tensor(out=sz[:], in0=ps_z[:], scalar=col(_BZ0, i),
                                                   in1=sgz[:], op0=OP.add, op1=OP.mult)

                    # ---- delta chain
                    et = sq.tile([128, T], BF16, tag="e1")
                    nc.scalar.activation(et[:], ps_d[:], AF.Exp,
                                         bias=col(_BDT2, i), scale=1.0)
                    dlt = sp.tile([128, T], BF16, tag="dlt")
                    nc.scalar.activation(dlt[:], et[:], AF.Ln, bias=1.0, scale=1.0)
                    du = sp.tile([128, T], BF16, tag="du")
                    nc.vector.tensor_tensor(out=du[:], in0=dlt[:], in1=u[:], op=OP.mult)

                    # ---- c_sb bf16 copy + state-major repack (broadcast source)
                    c_sb = sp.tile([128, T], BF16, tag="c_sb")
                    nc.scalar.copy(out=c_sb[:], in_=ps_c[:])
                    cps = c_sb[:].ap[0][0]
                    c3 = rp.tile([32, 4 * T], BF16, tag="c3")
                    c3ps = c3[:].ap[0][0]
                    nc.sync.dma_start(
                        out=bass.AP(tensor=c3[:].tensor, offset=c3[:].offset,
                                    ap=[[c3ps, 32], [T, 4], [1, T]]),
                        in_=bass.AP(tensor=c_sb[:].tensor, offset=c_sb[:].offset,
                                    ap=[[cps, 128], [1, T]]))

                    du_ap = du[:]
                    dut, duo, dups = du_ap.tensor, du_ap.offset, du_ap.ap[0][0]

                    y = sp.tile([128, T], BF16, tag="y")
                    for cc in range(NCK):
                        s0 = cc * 4
                        # DMA broadcasts into bf16 SBUF: each src partition row
                        # holds a full 4T state-chunk run (within-partition)
                        bbx = ck.tile([128, 4, T], BF16, tag="bbx")
                        b_src = bass.AP(tensor=b3[:].tensor,
                                        offset=b3[:].offset + cc * b3ps,
                                        ap=[[4 * b3ps, 2], [0, 64], [1, 4 * T]])
                        nc.sync.dma_start(out=bbx[:], in_=b_src)
                        cbx = ck.tile([128, 4, T], BF16, tag="cbx")
                        c_src = bass.AP(tensor=c3[:].tensor,
                                        offset=c3[:].offset + cc * c3ps,
                                        ap=[[4 * c3ps, 8], [0, 16], [1, 4 * T]])
                        nc.sync.dma_start(out=cbx[:], in_=c_src)

                        # dA = exp(delta * A_s), 4 slots
                        dAx = ck.tile([128, 4, T], BF16, tag="dAx")
                        for j in range(4):
                            nc.scalar.activation(dAx[:, j, :], dlt[:], AF.Exp, bias=0.0,
                                                 scale=col(_A0, i * 16 + s0 + j))

                        # b_in = du (bcast over slots) * bbx   [GpSimd]
                        binx = ck.tile([128, 4, T], BF16, tag="binx")
                        du_bc = bass.AP(tensor=dut, offset=duo,
                                        ap=[[dups, 128], [0, 4], [1, T]])
                        nc.gpsimd.tensor_tensor(out=binx[:], in0=du_bc, in1=bbx[:],
                                                op=OP.mult)

                        # scans (DVE)
                        hx = ck.tile([128, 4, T], BF16, tag="hx")
                        for j in range(4):
                            s = s0 + j
                            nc.vector.tensor_tensor_scan(
                                out=hx[:, j, :], data0=dAx[:, j, :], data1=binx[:, j, :],
                                initial=carry[:, i * 16 + s:i * 16 + s + 1],
                                op0=OP.mult, op1=OP.add)
                        nc.vector.tensor_copy(carry[:, i * 16 + s0:i * 16 + s0 + 4],
                                              hx[:, :, T - 1:T])

                        # m2 = h * cb  (DVE, one FD=2048 op)
                        m2x = ck1.tile([128, 4, T], BF16, tag="m2x")
                        nc.vector.tensor_tensor(out=m2x[:], in0=hx[:], in1=cbx[:],
                                                op=OP.mult)
                        # reduce: level1 on GpSimd, level2 + acc on DVE
                        r2 = ck1.tile([128, 2, T], BF16, tag="r2")
                        nc.gpsimd.tensor_tensor(out=r2[:], in0=m2x[:, 0:2, :],
                                                in1=m2x[:, 2:4, :], op=OP.add)
                        if cc == 0:
                            nc.vector.tensor_tensor(out=y[:], in0=r2[:, 0, :],
                                                    in1=r2[:, 1, :], op=OP.add)
                        else:
                            rc = ck1.tile([128, T], BF16, tag="rc")
                            nc.vector.tensor_tensor(out=rc[:], in0=r2[:, 0, :],
                                                    in1=r2[:, 1, :], op=OP.add)
                            nc.vector.tensor_tensor(out=y[:], in0=y[:], in1=rc[:],
                                                    op=OP.add)

                    # yf = (y + u*D) * sz
                    yq = sp.tile([128, T], BF16, tag="yq")
                    nc.vector.scalar_tensor_tensor(out=yq[:], in0=u[:], scalar=col(_D0, i),
                                                   in1=y[:], op0=OP.mult, op1=OP.add)
                    nc.gpsimd.tensor_tensor(out=yf[:, i, :], in0=yq[:], in1=sz[:],
                                            op=OP.mult)

                # ---- output projection partials: out[dm, t] = sum_i Wout_i.T @ yf_i
                for dmt in range(16):
                    ps_o = psO.tile([128, T], F32, tag="psO")
                    for i in range(NCT):
                        nc.tensor.matmul(ps_o[:],
                                         lhsT=w_wo[:, i, dmt * 128:(dmt + 1) * 128],
                                         rhs=yf[:, i, :],
                                         start=(i == 0), stop=(i == NCT - 1))
                    o_sb = sq.tile([128, T], BF16, tag="o_sb")
                    nc.scalar.copy(out=o_sb[:], in_=ps_o[:])
                    nc.sync.dma_start(
                        out=outp[dmt * 128:(dmt + 1) * 128, k * T:(k + 1) * T],
                        in_=o_sb[:])
    return nc


def _legalize_waits(nc):
    """This walrus build allows one sync-wait per instruction; split extras
    into standalone EventSemaphore waits on the same engine."""
    from concourse import mybir
    n = 0
    for fn in nc.m.functions:
        for blk in fn.blocks:
            newi = []
            for ins in blk.instructions:
                si = ins.sync_info
                if si is not None and si.on_wait is not None and len(si.on_wait) > 1:
                    for w in si.on_wait[:-1]:
                        ev = mybir.InstEventSemaphore(
                            name=f"W-{n}", ins=[], outs=[],
                            sync_info=mybir.SyncInfo(on_wait=[w], on_update=[]))
                        ev.engine = ins.engine
                        newi.append(ev)
                        n += 1
                    si.on_wait = [si.on_wait[-1]]
                newi.append(ins)
            blk.instructions = newi
    return n


def _prep_inputs(inputs, L):
    hs = np.asarray(inputs["hidden_states"], np.float32)
    Wx = np.asarray(inputs["Wx"], np.float32)
    bx = np.asarray(inputs["bx"], np.float32)
    Wz = np.asarray(inputs["Wz"], np.float32)
    bz = np.asarray(inputs["bz"], np.float32)
    conv_w = np.asarray(inputs["conv_w"], np.float32)
    conv_b = np.asarray(inputs["conv_b"], np.float32)
    WB = np.asarray(inputs["WB"], np.float32)
    WC = np.asarray(inputs["WC"], np.float32)
    Wdd = np.asarray(inputs["Wdt_down"], np.float32)
    Wdt = np.asarray(inputs["Wdt"], np.float32)
    bdt = np.asarray(inputs["bdt"], np.float32)
    A = -np.exp(np.asarray(inputs["A_log"], np.float32))
    D = np.asarray(inputs["D"], np.float32)
    Wout = np.asarray(inputs["Wout"], np.float32)

    in_maps = []
    for core in range(NCORES):
        bi, ci = core // NCH, core % NCH
        ch0 = ci * DIN_SH
        cs = slice(ch0, ch0 + DIN_SH)
        xs = slice(ci * DXB_SH, (ci + 1) * DXB_SH)
        consts = np.zeros((128, _NC), np.float32)
        convd = np.zeros((128, NCT * 4, 128), BF)
        for i in range(NCT):
            rows = slice(i * 128, (i + 1) * 128)
            consts[:, _A0 + i * 16:_A0 + (i + 1) * 16] = A[cs][rows]
            consts[:, _D0 + i] = D[cs][rows]
            consts[:, _CB0 + i] = conv_b[cs][rows]
            consts[:, _CBN + i] = -conv_b[cs][rows]
            consts[:, _BDT2 + i] = 2.0 * bdt[cs][rows]
            consts[:, _BZ0 + i] = bz[cs][rows]
            consts[:, _BZN + i] = -bz[cs][rows]
            consts[0:32, _BX0 + i] = bx[xs][i * 32:(i + 1) * 32]
            for j in range(DC):
                w4 = conv_w[cs, 0, j][rows]
                convd[np.arange(128), i * 4 + j, np.arange(128)] = w4.astype(BF)
        e32m = np.zeros((32, 128), BF)
        for c in range(128):
            e32m[(c // 64) * 16 + (c % 16), c] = 1
        # pack x and B projection weights: per ctile 64 cols = [x 32 | B 32]
        wxb = np.zeros((2 * DXB_SH, DM), np.float32)
        for i in range(NCT):
            wxb[i * 64:i * 64 + 32] = Wx[xs][i * 32:(i + 1) * 32]
            wxb[i * 64 + 32:i * 64 + 64] = WB[xs][i * 32:(i + 1) * 32]
        # streamed z/C weights: [NCT, 128p, 16kt, 128c] contiguous per ctile
        wzs = np.zeros((NCT, 128, 16, 128), BF)
        wcs = np.zeros((NCT, 128, 16, 128), BF)
        WzT = np.ascontiguousarray(Wz[cs].T)    # (DM, DIN_SH)
        WcT = np.ascontiguousarray(WC[cs].T)
        for i in range(NCT):
            blkz = WzT[:, i * 128:(i + 1) * 128].reshape(16, 128, 128)
            wzs[i] = blkz.transpose(1, 0, 2).astype(BF)
            blkc = WcT[:, i * 128:(i + 1) * 128].reshape(16, 128, 128)
            wcs[i] = blkc.transpose(1, 0, 2).astype(BF)
        in_maps.append({
            "hsT": np.ascontiguousarray(hs[bi, :L].T).astype(BF),
            "wxbT": np.ascontiguousarray(wxb.T).astype(BF),
            "wzS": wzs.reshape(NCT * 128, 16 * 128),
            "wcS": wcs.reshape(NCT * 128, 16 * 128),
            "wddT": np.ascontiguousarray(Wdd.T).astype(BF),
            "wdtT": np.ascontiguousarray(Wdt[cs].T).astype(BF),
            "woutT": np.ascontiguousarray(Wout[:, cs].T).astype(BF),
            "convd": convd.reshape(128, NCT * 4 * 128),
            "consts": consts,
            "e32": e32m,
        })
    return in_maps


def _install_profile_hook():
    """Make run_bass_kernel_spmd(trace=True) work: provide the
    antenv.axon_hooks registry the boot script looks for, backed by the
    ctypes NTFF start/stop calls into libaxon_pjrt.so."""
    import contextlib
    import ctypes
    import types

    import concourse.bass_utils as bu
    bu.upload_artifacts = lambda d: d  # no bucket in this container

    if "antenv.axon_hooks" not in sys.modules:
        mod = types.ModuleType("antenv.axon_hooks")
        _store = {}
        mod.set_axon_ntff_profile_hook = lambda h: _store.__setitem__("h", h)
        mod.get_axon_ntff_profile_hook = lambda: _store.get("h")
        sys.modules["antenv.axon_hooks"] = mod
        import antenv
        antenv.axon_hooks = mod

    from antenv.axon_hooks import get_axon_ntff_profile_hook, set_axon_ntff_profile_hook
    if get_axon_ntff_profile_hook() is not None:
        return
    lib = ctypes.CDLL("/opt/axon/libaxon_pjrt.so")
    if not hasattr(lib, "axon_start_nrt_profile"):
        return
    lib.axon_start_nrt_profile.argtypes = [ctypes.POINTER(ctypes.c_int64), ctypes.c_size_t]
    lib.axon_start_nrt_profile.restype = ctypes.c_int64
    lib.axon_stop_nrt_profile.argtypes = [ctypes.c_char_p]
    lib.axon_stop_nrt_profile.restype = ctypes.c_int64

    @contextlib.contextmanager
    def _hook(output_dir, device_ids):
        import jax
        jax.devices()
        if device_ids:
            ids = (ctypes.c_int64 * len(device_ids))(*device_ids)
            rc = lib.axon_start_nrt_profile(ids, len(device_ids))
        else:
            rc = lib.axon_start_nrt_profile(None, 0)
        if rc != 0:
            raise RuntimeError(f"axon_start_nrt_profile rc={rc}")
        try:
            yield
        finally:
            n = lib.axon_stop_nrt_profile(str(output_dir).encode())
            print(f"profile: {n} file(s) written to {output_dir}")

    set_axon_ntff_profile_hook(_hook)


def kernel(**inputs):
    from concourse.bass_utils import run_bass_kernel_spmd

    L, T = L_, 512
    nc = _build(L, T)
    _legalize_waits(nc)
    in_maps = _prep_inputs(inputs, L)
    trace = bool(int(os.environ.get("MAMBA_PROFILE", "0")))
    tmpdir = None
    if trace:
        import tempfile
        _install_profile_hook()
        tmpdir = tempfile.mkdtemp(prefix="mamba_trace_")
        kernel.last_trace_dir = tmpdir
    res = run_bass_kernel_spmd(nc, in_maps, core_ids=list(range(NCORES)), trace=trace,
                               tmpdir=tmpdir)
    if trace:
        kernel.last_exec_time_ns = res.exec_time_ns
        kernel.last_profile = res
    bout = np.asarray(inputs["bout"], np.float32)
    out = np.zeros((B_, L_, DM), np.float32)
    for bi in range(B_):
        acc = np.zeros((DM, L_), np.float32)
        for ci in range(NCH):
            acc += np.asarray(res.results[bi * NCH + ci]["outp"], np.float32)
        out[bi] = acc.T + bout[None, :]
    return out
